# revision 1
# baseline (speedup 1.0000x reference)
"""Trainium2 Bass kernel for nn_AttentionBlock (B=4, C=128, T=4096, K=64, V=128).

Sharding: 8 cores = 4 batches x 2 j-groups (pure data parallel over batch,
plus a split of the key/value axis j). The causal structure (only i <= j
survives the mask, softmax runs over the query axis i which is local to a
j-column) makes a j-split embarrassingly parallel up to a final sum of
partial read matrices, which the host performs.

j-tiles (128 wide) are interleaved between the two j-group cores so the
triangular live region is load-balanced and, crucially, the number of live
512-wide i-chunks per local j-tile index is IDENTICAL on every core
(d_k = k//2 + 1), so one SPMD program serves all 8 cores; only input data
(x slice, gathered j-columns, additive mask tiles) differs per core.

Device computes outT = read^T partial [V=128, T] directly (the output needs
[B, C+V, T]; rows C: are read^T and rows :C are x itself, so the x
passthrough costs nothing on device).
"""

import numpy as np

_B, _C, _T = 4, 128, 4096
_K, _V = 64, 128
_JT = 16          # local 128-wide j tiles per core -> 2048 local j columns
_CH = 512         # i-chunk width (one PSUM bank in fp32)
_ICH = _T // _CH  # 8 i-chunks

_NEG = -1.0e30    # effective -inf for the causal mask (exp -> 0 exactly)

_cache = {}


def _build_nc():
    from contextlib import ExitStack

    import concourse.tile as tile
    from concourse import bacc, mybir
    from concourse.masks import make_identity

    f32 = mybir.dt.float32
    bf16 = mybir.dt.bfloat16
    AF = mybir.ActivationFunctionType

    nc = bacc.Bacc("TRN2", target_bir_lowering=False)

    xb_d = nc.dram_tensor("xb", [_C, _T], f32, kind="ExternalInput")
    xj_d = nc.dram_tensor("xj", [_C, _JT * 128], f32, kind="ExternalInput")
    wq_d = nc.dram_tensor("wq", [_C, _K], f32, kind="ExternalInput")
    wk_d = nc.dram_tensor("wk", [_C, _K], f32, kind="ExternalInput")
    wv_d = nc.dram_tensor("wv", [_C, _V], f32, kind="ExternalInput")
    bq_d = nc.dram_tensor("bq", [_K, 1], f32, kind="ExternalInput")
    bk_d = nc.dram_tensor("bk", [_K, 1], f32, kind="ExternalInput")
    bv_d = nc.dram_tensor("bv", [1, _V], f32, kind="ExternalInput")
    mk_d = nc.dram_tensor("mask", [2, 128, _CH], f32, kind="ExternalInput")
    out_d = nc.dram_tensor("out", [_V, _T], f32, kind="ExternalOutput")

    with tile.TileContext(nc) as tc, ExitStack() as ctx:
        singles = ctx.enter_context(tc.tile_pool(name="singles", bufs=1))
        work = ctx.enter_context(tc.tile_pool(name="work", bufs=3))
        small = ctx.enter_context(tc.tile_pool(name="small", bufs=4))
        psum = ctx.enter_context(tc.tile_pool(name="psum", bufs=2, space="PSUM"))

        # ---------------- load + cast to bf16 ----------------
        xb_bf = singles.tile([_C, _T], bf16)
        xj_bf = singles.tile([_C, _JT * 128], bf16)
        for c in range(_ICH):
            t = work.tile([_C, _CH], f32, tag="ld")
            nc.sync.dma_start(out=t, in_=xb_d[:, c * _CH:(c + 1) * _CH])
            nc.vector.tensor_copy(xb_bf[:, c * _CH:(c + 1) * _CH], t)
        for c in range(_JT * 128 // _CH):
            t = work.tile([_C, _CH], f32, tag="ld")
            nc.sync.dma_start(out=t, in_=xj_d[:, c * _CH:(c + 1) * _CH])
            nc.vector.tensor_copy(xj_bf[:, c * _CH:(c + 1) * _CH], t)

        wq_bf = singles.tile([_C, _K], bf16)
        wk_bf = singles.tile([_C, _K], bf16)
        wv_bf = singles.tile([_C, _V], bf16)
        for d_, t_ in ((wq_d, wq_bf), (wk_d, wk_bf), (wv_d, wv_bf)):
            w = d_.shape[1]
            tmp = work.tile([_C, _V], f32, tag="wld")
            nc.sync.dma_start(out=tmp[:, :w], in_=d_[:])
            nc.vector.tensor_copy(t_, tmp[:, :w])

        bq_s = singles.tile([_K, 1], f32)
        nc.sync.dma_start(out=bq_s, in_=bq_d[:])
        bk_s = singles.tile([_K, 1], f32)
        nc.sync.dma_start(out=bk_s, in_=bk_d[:])
        bv_s = singles.tile([1, _V], f32)
        nc.sync.dma_start(out=bv_s, in_=bv_d[:])
        bv_bf = singles.tile([1, _V], bf16)
        nc.vector.tensor_copy(bv_bf, bv_s)
        ones_bf = singles.tile([1, 128], bf16)
        nc.vector.memset(ones_bf, 1.0)

        mask_f = singles.tile([128, 2, _CH], f32)
        mask_bf = singles.tile([128, 2, _CH], bf16)
        for m in range(2):
            nc.sync.dma_start(out=mask_f[:, m, :], in_=mk_d[m])
        nc.vector.tensor_copy(mask_bf, mask_f)

        id_bf = singles.tile([128, 128], bf16)
        make_identity(nc, id_bf[:])

        # ---------------- projections ----------------
        # qt[kk, i] = sum_c Wq[c, kk] * x[c, i] + bq[kk]   (Q^T, [64, T])
        qt_bf = singles.tile([_K, _T], bf16)
        for c in range(_ICH):
            ps = psum.tile([128, 2048], f32, tag="ps")
            nc.tensor.matmul(ps[0:_K, 0:_CH], wq_bf,
                             xb_bf[:, c * _CH:(c + 1) * _CH],
                             start=True, stop=True)
            nc.vector.tensor_scalar_add(qt_bf[:, c * _CH:(c + 1) * _CH],
                                        ps[0:_K, 0:_CH], bq_s[:])
        # kt[kk, jl] over this core's gathered j columns ([64, 2048])
        kt_bf = singles.tile([_K, _JT * 128], bf16)
        for c in range(_JT * 128 // _CH):
            ps = psum.tile([128, 2048], f32, tag="ps")
            nc.tensor.matmul(ps[0:_K, 0:_CH], wk_bf,
                             xj_bf[:, c * _CH:(c + 1) * _CH],
                             start=True, stop=True)
            nc.vector.tensor_scalar_add(kt_bf[:, c * _CH:(c + 1) * _CH],
                                        ps[0:_K, 0:_CH], bk_s[:])
        # v[jl, v] = sum_c x[c, jl] * Wv[c, v] + bv[v]   ([128, V] per j-tile)
        v_f32 = singles.tile([128, _JT, _V], f32)
        for k in range(_JT):
            ps = psum.tile([128, 2048], f32, tag="ps")
            nc.tensor.matmul(ps[:, 0:_V], xj_bf[:, k * 128:(k + 1) * 128],
                             wv_bf, start=True, stop=False)
            nc.tensor.matmul(ps[:, 0:_V], ones_bf, bv_bf,
                             start=False, stop=True)
            nc.vector.tensor_copy(v_f32[:, k, :], ps[:, 0:_V])

        # ---------------- attention ----------------
        # Per local j-tile k (descending so read chunks unlock early):
        #   logits^T[jl, i] for live i-chunks only (d = k//2 + 1 of them),
        #   additive causal mask on the diagonal chunk (via PE identity
        #   matmul into the same PSUM accumulation group),
        #   e = exp(logits/8) via ScalarE with row-sum accum,
        #   vs[jl, :] = (V[jl, :]) / s[jl]  in bf16.
        # After tile k (k even): read i-chunk c = k//2 is fully determined:
        #   outT[v, i] += sum_jl vs[jl, v] * e[jl, i] over j-tiles >= 2c.
        e_all = singles.tile([128, _JT, _T], bf16)
        vs_bf = singles.tile([128, _JT, _V], bf16)

        def emit_read_chunk(c):
            ks = list(range(2 * c, _JT))
            ps = psum.tile([128, 2048], f32, tag="ps")
            for i, k in enumerate(ks):
                nc.tensor.matmul(ps[0:_V, 0:_CH], vs_bf[:, k, :],
                                 e_all[:, k, c * _CH:(c + 1) * _CH],
                                 start=(i == 0), stop=(i == len(ks) - 1))
            ot = work.tile([_V, _CH], f32, tag="osb")
            nc.vector.tensor_copy(ot, ps[0:_V, 0:_CH])
            nc.sync.dma_start(out=out_d[:, c * _CH:(c + 1) * _CH], in_=ot)

        for k in range(_JT - 1, -1, -1):
            d = k // 2 + 1
            accs = []
            for g0 in range(0, d, 4):
                g1 = min(g0 + 4, d)
                ps = psum.tile([128, 2048], f32, tag="ps")
                for c in range(g0, g1):
                    diag = (c == d - 1)
                    nc.tensor.matmul(ps[:, (c - g0) * _CH:(c - g0 + 1) * _CH],
                                     kt_bf[:, k * 128:(k + 1) * 128],
                                     qt_bf[:, c * _CH:(c + 1) * _CH],
                                     start=True, stop=not diag)
                    if diag:
                        nc.tensor.matmul(
                            ps[:, (c - g0) * _CH:(c - g0 + 1) * _CH],
                            id_bf, mask_bf[:, k % 2, :],
                            start=False, stop=True)
                acc = small.tile([128, 1], f32, tag="acc")
                nc.scalar.activation(out=e_all[:, k, g0 * _CH:g1 * _CH],
                                     in_=ps[:, 0:(g1 - g0) * _CH],
                                     func=AF.Exp, scale=0.125, accum_out=acc)
                accs.append(acc)
            if len(accs) == 1:
                s_t = accs[0]
            else:
                s_t = small.tile([128, 1], f32, tag="s")
                nc.vector.tensor_add(s_t, accs[0], accs[1])
            rs = small.tile([128, 1], f32, tag="rs")
            nc.vector.reciprocal(rs, s_t)
            nc.vector.tensor_scalar_mul(vs_bf[:, k, :], v_f32[:, k, :], rs)

            if k % 2 == 0:
                emit_read_chunk(k // 2)

    nc.compile()
    return nc


def _get_nc():
    if "nc" not in _cache:
        _cache["nc"] = _build_nc()
    return _cache["nc"]


def _masks(g):
    """Additive causal-mask tiles for a core in j-group g.

    Tile m (= local j-tile parity) masks the diagonal 512-wide i-chunk of
    every local j-tile with that parity: entry [p, ii] is live iff
    global_i <= global_j, i.e. ii <= (j0 - i0) + p with j0 - i0 = 128*g + 256*m.
    """
    m = np.zeros((2, 128, _CH), np.float32)
    p = np.arange(128)[:, None]
    ii = np.arange(_CH)[None, :]
    for parity in range(2):
        o = 128 * g + 256 * parity
        m[parity] = np.where(ii <= o + p, 0.0, _NEG)
    return m


def kernel(**inputs):
    from concourse.bass_utils import run_bass_kernel_spmd

    x = np.ascontiguousarray(np.asarray(inputs["x"], dtype=np.float32))
    Wq = np.ascontiguousarray(np.asarray(inputs["Wq"], dtype=np.float32))
    Wk = np.ascontiguousarray(np.asarray(inputs["Wk"], dtype=np.float32))
    Wv = np.ascontiguousarray(np.asarray(inputs["Wv"], dtype=np.float32))
    bq = np.ascontiguousarray(
        np.asarray(inputs["bq"], dtype=np.float32).reshape(_K, 1))
    bk = np.ascontiguousarray(
        np.asarray(inputs["bk"], dtype=np.float32).reshape(_K, 1))
    bv = np.ascontiguousarray(
        np.asarray(inputs["bv"], dtype=np.float32).reshape(1, _V))

    nc = _get_nc()
    in_maps = []
    for core in range(8):
        b, g = divmod(core, 2)
        # this core's j columns: tiles {2k+g}, i.e. starts 256k + 128g
        cols = ((np.arange(_JT) * 256 + 128 * g)[:, None]
                + np.arange(128)[None, :]).ravel()
        in_maps.append({
            "xb": np.ascontiguousarray(x[b]),
            "xj": np.ascontiguousarray(x[b][:, cols]),
            "wq": Wq, "wk": Wk, "wv": Wv,
            "bq": bq, "bk": bk, "bv": bv,
            "mask": _masks(g),
        })

    trace = bool(_cache.get("trace"))
    res = run_bass_kernel_spmd(nc, in_maps, core_ids=list(range(8)),
                               trace=trace)
    _cache["last_result"] = res

    parts = [r["out"] for r in res.results]
    out = np.empty((_B, _C + _V, _T), np.float32)
    for b in range(_B):
        out[b, :_C] = x[b]
        out[b, _C:] = parts[2 * b] + parts[2 * b + 1]
    return out



# revision 5
# speedup vs baseline: 1.2584x; 1.2584x over previous
"""Trainium2 Bass kernel for nn_AttentionBlock (B=4, C=128, T=4096, K=64, V=128).

Sharding: 8 cores = 4 batches x 2 j-groups. Core (b, g) owns global j-tiles
{2k+g : k=0..15} of batch b. Softmax runs over the query axis i (local to a
j column), so a j-split is embarrassingly parallel up to a final sum of the
partial read matrices, done on the host.

Per local j-tile k the live i region (i <= j) is covered by a uniform strip
i in [0, (2k+2)*128): the ceil over both j-groups, so one SPMD program fits
all cores; a per-core additive mask on the strip's last 256 columns encodes
both the causal triangle and the g=0 overhang (mask content is data, so it
may differ per core while the program stays uniform).

Device computes outT = partial read^T [V=128, T] (rows :C of the final
output are x itself and are stitched on the host, which also sums the two
j-group partials per batch).

Schedule per strip k = 15..0:
  PE:  logits chunks (<=1536 cols = 3 PSUM banks, 2 rotating) with the
       identity-matmul mask fold on the diagonal chunk; a tiny V projection
       (1 bank); read-pass bursts at pass boundaries (1 bank).
  ACT: exp(chunk * 0.125) -> e_all (bf16, SBUF) with row-sum accumulators.
  DVE: bias adds for Q^T/K^T, s combine, reciprocal, vs = v/s, read drains.
  GPS: f32 -> bf16 casts of x / xj (feed the projections).
"""

import numpy as np

_B, _C, _T = 4, 128, 4096
_K, _V = 64, 128
_JT = 16           # local j-tiles per core (128 wide) -> 2048 local j columns
_LG = 1536         # logits PSUM chunk width (3 banks)
_NEG = -1.0e30

# read passes: (i0, i1, kmin) — pass covers out cols [i0, i1), summing
# strips k >= kmin (the strips whose width exceeds i0)
_PASSES = [(3584, 4096, 14), (3072, 3584, 12), (2560, 3072, 10),
           (2048, 2560, 8), (1536, 2048, 6), (1024, 1536, 4),
           (512, 1024, 2), (256, 512, 1), (0, 256, 0)]


def _W(k):
    return (2 * k + 2) * 128


def _S(k):
    return 128 * k * (k + 1)


_ETOT = _S(_JT)    # 34816 columns of e per core

_cache = {}


def _build_nc():
    from contextlib import ExitStack

    import concourse.tile as tile
    from concourse import bacc, mybir
    from concourse.masks import make_identity

    f32 = mybir.dt.float32
    bf16 = mybir.dt.bfloat16
    AF = mybir.ActivationFunctionType

    nc = bacc.Bacc("TRN2", target_bir_lowering=False)

    xb_d = nc.dram_tensor("xb", [_C, _T], f32, kind="ExternalInput")
    xj_d = nc.dram_tensor("xj", [_C, _JT * 128], f32, kind="ExternalInput")
    wq_d = nc.dram_tensor("wq", [_C, _K], f32, kind="ExternalInput")
    wk_d = nc.dram_tensor("wk", [_C, _K], f32, kind="ExternalInput")
    wv_d = nc.dram_tensor("wv", [_C, _V], f32, kind="ExternalInput")
    bq_d = nc.dram_tensor("bq", [_K, 1], f32, kind="ExternalInput")
    bk_d = nc.dram_tensor("bk", [_K, 1], f32, kind="ExternalInput")
    bv_d = nc.dram_tensor("bv", [1, _V], f32, kind="ExternalInput")
    mk_d = nc.dram_tensor("mask", [128, 256], f32, kind="ExternalInput")
    out_d = nc.dram_tensor("out", [_V, _T], f32, kind="ExternalOutput")

    with tile.TileContext(nc) as tc, ExitStack() as ctx:
        singles = ctx.enter_context(tc.tile_pool(name="singles", bufs=1))
        work = ctx.enter_context(tc.tile_pool(name="work", bufs=2))
        small = ctx.enter_context(tc.tile_pool(name="small", bufs=8))
        lg = ctx.enter_context(tc.tile_pool(name="lg", bufs=2, space="PSUM"))
        rd = ctx.enter_context(tc.tile_pool(name="rd", bufs=1, space="PSUM"))
        vv = ctx.enter_context(tc.tile_pool(name="vv", bufs=1, space="PSUM"))

        # ---------------- input DMA + bf16 casts ----------------
        xb_f = singles.tile([_C, _T], f32)
        xb_bf = singles.tile([_C, _T], bf16)
        xj_f = singles.tile([_C, _JT * 128], f32)
        xj_bf = singles.tile([_C, _JT * 128], bf16)

        def load_xb(c0, c1):
            nc.sync.dma_start(out=xb_f[:, c0:c1], in_=xb_d[:, c0:c1])
            nc.gpsimd.tensor_copy(xb_bf[:, c0:c1], xb_f[:, c0:c1])

        def load_xj(c0, c1):
            nc.sync.dma_start(out=xj_f[:, c0:c1], in_=xj_d[:, c0:c1])
            nc.gpsimd.tensor_copy(xj_bf[:, c0:c1], xj_f[:, c0:c1])

        # order: K^T hi tiles first, then qt hi, then qt lo, then the rest
        load_xj(1536, 2048)
        load_xb(3072, 3584)
        load_xb(3584, 4096)

        wq_s = singles.tile([_C, _K], f32)
        nc.sync.dma_start(out=wq_s, in_=wq_d[:])
        wk_s = singles.tile([_C, _K], f32)
        nc.sync.dma_start(out=wk_s, in_=wk_d[:])
        wv_s = singles.tile([_C, _V], f32)
        nc.sync.dma_start(out=wv_s, in_=wv_d[:])
        wq_bf = singles.tile([_C, _K], bf16)
        nc.vector.tensor_copy(wq_bf, wq_s)
        wk_bf = singles.tile([_C, _K], bf16)
        nc.vector.tensor_copy(wk_bf, wk_s)
        wv_bf = singles.tile([_C, _V], bf16)
        nc.vector.tensor_copy(wv_bf, wv_s)

        bq_s = singles.tile([_K, 1], f32)
        nc.sync.dma_start(out=bq_s, in_=bq_d[:])
        bk_s = singles.tile([_K, 1], f32)
        nc.sync.dma_start(out=bk_s, in_=bk_d[:])
        bv_s = singles.tile([1, _V], f32)
        nc.sync.dma_start(out=bv_s, in_=bv_d[:])
        bv_bf = singles.tile([1, _V], bf16)
        nc.vector.tensor_copy(bv_bf, bv_s)
        ones_bf = singles.tile([1, 128], bf16)
        nc.vector.memset(ones_bf, 1.0)

        mask_f = singles.tile([128, 256], f32)
        nc.sync.dma_start(out=mask_f, in_=mk_d[:])
        mask_bf = singles.tile([128, 256], bf16)
        nc.vector.tensor_copy(mask_bf, mask_f)

        id_bf = singles.tile([128, 128], bf16)
        make_identity(nc, id_bf[:])

        for c0, c1 in ((0, 512), (512, 1024), (1024, 1536)):
            load_xb(c0, c1)
        load_xj(0, 512)
        load_xj(512, 1024)
        load_xj(1024, 1536)
        for c0, c1 in ((1536, 2048), (2048, 2560), (2560, 3072)):
            load_xb(c0, c1)

        # ---------------- Q^T / K^T projections (borrow lg pool) --------
        qt_bf = singles.tile([_K, _T], bf16)
        kt_bf = singles.tile([_K, _JT * 128], bf16)

        def proj(dst, w_bf, src_bf, b_s, a, b):
            # matmul outputs must stay within one PSUM bank (512 f32)
            ps = lg.tile([128, _LG], f32, tag="lg")
            for o in range(0, b - a, 512):
                w = min(o + 512, b - a) - o
                nc.tensor.matmul(ps[0:_K, o:o + w], w_bf,
                                 src_bf[:, a + o:a + o + w],
                                 start=True, stop=True)
            nc.vector.tensor_scalar_add(dst[:, a:b], ps[0:_K, 0:b - a], b_s[:])

        proj(kt_bf, wk_bf, xj_bf, bk_s, 1536, 2048)   # K^T tiles 12..15
        proj(qt_bf, wq_bf, xb_bf, bq_s, 3072, 4096)   # qt hi
        proj(qt_bf, wq_bf, xb_bf, bq_s, 0, 1536)
        proj(qt_bf, wq_bf, xb_bf, bq_s, 1536, 3072)
        proj(kt_bf, wk_bf, xj_bf, bk_s, 0, 1536)      # K^T tiles 0..11

        # ---------------- attention ----------------
        e_all = singles.tile([128, _ETOT], bf16)
        vs_bf = singles.tile([128, _JT, _V], bf16)

        for k in range(_JT - 1, -1, -1):
            W = _W(k)
            S = _S(k)
            n_k = -(-W // _LG)
            kt_k = kt_bf[:, k * 128:(k + 1) * 128]

            # V projection for this tile (1-bank PSUM, consumed by vs mul)
            v_ps = vv.tile([128, _V], f32, tag="vv")
            nc.tensor.matmul(v_ps, xj_bf[:, k * 128:(k + 1) * 128], wv_bf,
                             start=True, stop=False)
            nc.tensor.matmul(v_ps, ones_bf, bv_bf, start=False, stop=True)

            accs = []
            # diagonal chunk first (its qt cols are DMA'd earliest for hi k)
            order = [n_k - 1] + list(range(n_k - 1))
            for c in order:
                a, b = c * _LG, min((c + 1) * _LG, W)
                ps = lg.tile([128, _LG], f32, tag="lg")
                diag = c == n_k - 1
                m0 = (b - a) - 256
                for o in range(0, b - a, 512):
                    w = min(o + 512, b - a) - o
                    last_bank = diag and (o + w == b - a)
                    nc.tensor.matmul(ps[:, o:o + w], kt_k,
                                     qt_bf[:, a + o:a + o + w],
                                     start=True, stop=not last_bank)
                if diag:
                    nc.tensor.matmul(ps[:, m0:m0 + 256], id_bf, mask_bf,
                                     start=False, stop=True)
                acc = small.tile([128, 1], f32, tag="acc")
                nc.scalar.activation(out=e_all[:, S + a:S + b],
                                     in_=ps[:, 0:b - a],
                                     func=AF.Exp, scale=0.125, accum_out=acc)
                accs.append(acc)

            s_t = accs[0]
            for extra in accs[1:]:
                s2 = small.tile([128, 1], f32, tag="s")
                nc.vector.tensor_add(s2, s_t, extra)
                s_t = s2
            rs = small.tile([128, 1], f32, tag="rs")
            nc.vector.reciprocal(rs, s_t)
            nc.vector.tensor_scalar_mul(vs_bf[:, k, :], v_ps, rs)

            # read passes that become complete at this strip
            for (i0, i1, kmin) in _PASSES:
                if kmin != k:
                    continue
                w_pass = i1 - i0
                ps_r = rd.tile([128, 512], f32, tag="rd")
                for kk in range(_JT - 1, k - 1, -1):
                    w = min(_W(kk), i1) - i0
                    nc.tensor.matmul(ps_r[0:_V, 0:w], vs_bf[:, kk, :],
                                     e_all[:, _S(kk) + i0:_S(kk) + i0 + w],
                                     start=(kk == _JT - 1), stop=(kk == k))
                ot = work.tile([_V, 512], f32, tag="osb")
                nc.vector.tensor_copy(ot[:, 0:w_pass], ps_r[0:_V, 0:w_pass])
                nc.sync.dma_start(out=out_d[:, i0:i1], in_=ot[:, 0:w_pass])

    nc.compile()
    return nc


def _get_nc():
    if "nc" not in _cache:
        _cache["nc"] = _build_nc()
    return _cache["nc"]


def _masks(g):
    """Additive mask for the last 256 columns of every strip.

    Strip for local tile k covers i in [0, (2k+2)*128); its last 256
    columns are i = 2k*128 + u, u in [0, 256). Partition p holds global
    j = (2k+g)*128 + p, so live (i <= j) iff u <= 128*g + p.
    """
    m = np.zeros((128, 256), np.float32)
    p = np.arange(128)[:, None]
    u = np.arange(256)[None, :]
    m[:] = np.where(u <= 128 * g + p, 0.0, _NEG)
    return m


def kernel(**inputs):
    from concourse.bass_utils import run_bass_kernel_spmd

    x = np.ascontiguousarray(np.asarray(inputs["x"], dtype=np.float32))
    Wq = np.ascontiguousarray(np.asarray(inputs["Wq"], dtype=np.float32))
    Wk = np.ascontiguousarray(np.asarray(inputs["Wk"], dtype=np.float32))
    Wv = np.ascontiguousarray(np.asarray(inputs["Wv"], dtype=np.float32))
    bq = np.ascontiguousarray(
        np.asarray(inputs["bq"], dtype=np.float32).reshape(_K, 1))
    bk = np.ascontiguousarray(
        np.asarray(inputs["bk"], dtype=np.float32).reshape(_K, 1))
    bv = np.ascontiguousarray(
        np.asarray(inputs["bv"], dtype=np.float32).reshape(1, _V))

    nc = _get_nc()
    in_maps = []
    for core in range(8):
        b, g = divmod(core, 2)
        # this core's j columns: global tiles {2k+g}, i.e. starts 256k+128g
        cols = ((np.arange(_JT) * 256 + 128 * g)[:, None]
                + np.arange(128)[None, :]).ravel()
        in_maps.append({
            "xb": np.ascontiguousarray(x[b]),
            "xj": np.ascontiguousarray(x[b][:, cols]),
            "wq": Wq, "wk": Wk, "wv": Wv,
            "bq": bq, "bk": bk, "bv": bv,
            "mask": _masks(g),
        })

    trace = bool(_cache.get("trace"))
    res = run_bass_kernel_spmd(nc, in_maps, core_ids=list(range(8)),
                               trace=trace)
    _cache["last_result"] = res

    parts = [r["out"] for r in res.results]
    out = np.empty((_B, _C + _V, _T), np.float32)
    for b in range(_B):
        out[b, :_C] = x[b]
        out[b, _C:] = parts[2 * b] + parts[2 * b + 1]
    return out


# revision 8
# speedup vs baseline: 1.3081x; 1.0395x over previous
"""Trainium2 Bass kernel for nn_AttentionBlock (B=4, C=128, T=4096, K=64, V=128).

Sharding: 8 cores = 4 batches x 2 j-groups. Core (b, g) owns global j-tiles
{2k+g : k=0..15} of batch b. Softmax runs over the query axis i (local to a
j column), so a j-split is embarrassingly parallel up to a final sum of the
partial read matrices, done on the host.

Per local j-tile k the live i region (i <= j) is covered by a uniform strip
i in [0, (2k+2)*128): the ceil over both j-groups, so one SPMD program fits
all cores; a per-core additive mask on the strip's last 256 columns encodes
both the causal triangle and the g=0 overhang (mask content is data, so it
may differ per core while the program stays uniform).

Device computes outT = partial read^T [V=128, T] (rows :C of the final
output are x itself and are stitched on the host, which also sums the two
j-group partials per batch).

Key engine choices:
  - Projections run in float32r straight from the f32 DMA tiles (no cast
    pass at all); moving free dims kept >= 256 (Wv/bv padded) for full
    f32r rate.
  - PE per strip: logits in 512-col sub-matmuls (PSUM-bank ISA limit) into
    1536-col chunks (2 rotating 3-bank tiles), identity-matmul mask fold on
    the diagonal chunk, tiny V projection, read-pass bursts delayed by one
    strip so ACT never waits on them.
  - ACT: exp(chunk * 0.125) -> e_all bf16 with row-sum accumulators.
  - DVE: Q^T/K^T bias adds, s combine, reciprocal, vs = v/s, read drains.
"""

import numpy as np

_B, _C, _T = 4, 128, 4096
_K, _V = 64, 128
_JT = 16           # local j-tiles per core (128 wide) -> 2048 local j columns
_LG = 1536         # logits PSUM chunk width (3 banks)
_NEG = -1.0e30

# read passes: (i0, i1, kmin) — pass covers out cols [i0, i1), summing
# strips k >= kmin (the strips whose width exceeds i0). The burst for a
# pass is emitted during strip kmin-1 (vs[kmin] exists by then); the final
# (0,256) pass is split so only its k'=0 matmul trails the last exp.
_PASSES = [(3584, 4096, 14), (3072, 3584, 12), (2560, 3072, 10),
           (2048, 2560, 8), (1536, 2048, 6), (1024, 1536, 4),
           (512, 1024, 2), (256, 512, 1), (0, 256, 0)]


def _W(k):
    return (2 * k + 2) * 128


def _S(k):
    return 128 * k * (k + 1)


_ETOT = _S(_JT)    # 34816 columns of e per core

_cache = {}


def _build_nc():
    from contextlib import ExitStack

    import concourse.tile as tile
    from concourse import bacc, mybir
    from concourse.masks import make_identity

    f32 = mybir.dt.float32
    f32r = mybir.dt.float32r
    bf16 = mybir.dt.bfloat16
    AF = mybir.ActivationFunctionType

    nc = bacc.Bacc("TRN2", target_bir_lowering=False)

    xb_d = nc.dram_tensor("xb", [_C, _T], f32r, kind="ExternalInput")
    xj_d = nc.dram_tensor("xj", [_C, _JT * 128], f32r, kind="ExternalInput")
    wq_d = nc.dram_tensor("wq", [_C, _K], f32r, kind="ExternalInput")
    wk_d = nc.dram_tensor("wk", [_C, _K], f32r, kind="ExternalInput")
    wv_d = nc.dram_tensor("wv", [_C, 256], f32r, kind="ExternalInput")
    bq_d = nc.dram_tensor("bq", [_K, 1], f32, kind="ExternalInput")
    bk_d = nc.dram_tensor("bk", [_K, 1], f32, kind="ExternalInput")
    bv_d = nc.dram_tensor("bv", [1, 256], f32r, kind="ExternalInput")
    on_d = nc.dram_tensor("ones", [1, 128], f32r, kind="ExternalInput")
    mk_d = nc.dram_tensor("mask", [128, 256], f32, kind="ExternalInput")
    out_d = nc.dram_tensor("out", [_V, _T], f32, kind="ExternalOutput")

    with tile.TileContext(nc) as tc, ExitStack() as ctx:
        singles = ctx.enter_context(tc.tile_pool(name="singles", bufs=1))
        work = ctx.enter_context(tc.tile_pool(name="work", bufs=2))
        small = ctx.enter_context(tc.tile_pool(name="small", bufs=8))
        lg = ctx.enter_context(tc.tile_pool(name="lg", bufs=2, space="PSUM"))
        rd = ctx.enter_context(tc.tile_pool(name="rd", bufs=1, space="PSUM"))
        vv = ctx.enter_context(tc.tile_pool(name="vv", bufs=1, space="PSUM"))

        # ---------------- input DMA ----------------
        xb_f = singles.tile([_C, _T], f32r)
        xj_f = singles.tile([_C, _JT * 128], f32r)

        def load(dst, src, c0, c1):
            nc.sync.dma_start(out=dst[:, c0:c1], in_=src[:, c0:c1])

        load(xj_f, xj_d, 1536, 2048)          # K^T/V tiles 12..15
        load(xb_f, xb_d, 3072, 4096)          # qt hi

        wq_s = singles.tile([_C, _K], f32r)
        nc.sync.dma_start(out=wq_s, in_=wq_d[:])
        wk_s = singles.tile([_C, _K], f32r)
        nc.sync.dma_start(out=wk_s, in_=wk_d[:])
        wv_p = singles.tile([_C, 256], f32r)   # padded to 256 free for f32r
        nc.sync.dma_start(out=wv_p, in_=wv_d[:])
        bq_s = singles.tile([_K, 1], f32)
        nc.sync.dma_start(out=bq_s, in_=bq_d[:])
        bk_s = singles.tile([_K, 1], f32)
        nc.sync.dma_start(out=bk_s, in_=bk_d[:])
        bv_p = singles.tile([1, 256], f32r)
        nc.sync.dma_start(out=bv_p, in_=bv_d[:])
        ones_s = singles.tile([1, 128], f32r)
        nc.sync.dma_start(out=ones_s, in_=on_d[:])

        mask_f = singles.tile([128, 256], f32)
        nc.sync.dma_start(out=mask_f, in_=mk_d[:])
        mask_bf = singles.tile([128, 256], bf16)
        nc.vector.tensor_copy(mask_bf, mask_f)

        id_bf = singles.tile([128, 128], bf16)
        make_identity(nc, id_bf[:])

        load(xb_f, xb_d, 0, 1536)
        load(xb_f, xb_d, 1536, 3072)
        load(xj_f, xj_d, 0, 1536)

        # ---------------- Q^T / K^T projections (borrow lg pool) --------
        qt_bf = singles.tile([_K, _T], bf16)
        kt_bf = singles.tile([_K, _JT * 128], bf16)

        def proj(dst, w_s, src_f, b_s, a, b):
            # matmul outputs must stay within one PSUM bank (512 f32)
            ps = lg.tile([128, _LG], f32, tag="lg")
            for o in range(0, b - a, 512):
                w = min(o + 512, b - a) - o
                nc.tensor.matmul(ps[0:_K, o:o + w], w_s[:],
                                 src_f[:, a + o:a + o + w],
                                 start=True, stop=True)
            nc.vector.tensor_scalar_add(dst[:, a:b], ps[0:_K, 0:b - a], b_s[:])

        proj(kt_bf, wk_s, xj_f, bk_s, 1536, 2048)   # K^T tiles 12..15
        proj(qt_bf, wq_s, xb_f, bq_s, 3072, 4096)   # qt hi
        proj(qt_bf, wq_s, xb_f, bq_s, 0, 1536)
        proj(qt_bf, wq_s, xb_f, bq_s, 1536, 3072)
        proj(kt_bf, wk_s, xj_f, bk_s, 0, 1536)      # K^T tiles 0..11

        # ---------------- attention ----------------
        e_all = singles.tile([128, _ETOT], bf16)
        vs_bf = singles.tile([128, _JT, _V], bf16)

        def read_pass(i0, i1, kmin, klo=None):
            # accumulate read^T over strips kmin..15 for out cols [i0, i1)
            ps_r = rd.tile([128, 512], f32, tag="rd")
            for kk in range(_JT - 1, (klo if klo is not None else kmin) - 1,
                            -1):
                w = min(_W(kk), i1) - i0
                nc.tensor.matmul(ps_r[0:_V, 0:w], vs_bf[:, kk, :],
                                 e_all[:, _S(kk) + i0:_S(kk) + i0 + w],
                                 start=(kk == _JT - 1), stop=(kk == kmin))
            return ps_r

        def drain_pass(ps_r, i0, i1):
            ot = work.tile([_V, 512], f32, tag="osb")
            nc.vector.tensor_copy(ot[:, 0:i1 - i0], ps_r[0:_V, 0:i1 - i0])
            nc.sync.dma_start(out=out_d[:, i0:i1], in_=ot[:, 0:i1 - i0])

        pending = []
        for k in range(_JT - 1, -1, -1):
            W = _W(k)
            S = _S(k)
            n_k = -(-W // _LG)
            kt_k = kt_bf[:, k * 128:(k + 1) * 128]

            accs = []
            # diagonal chunk first (its qt cols are DMA'd earliest for hi k)
            order = [n_k - 1] + list(range(n_k - 1))
            for c in order:
                a, b = c * _LG, min((c + 1) * _LG, W)
                ps = lg.tile([128, _LG], f32, tag="lg")
                diag = c == n_k - 1
                m0 = (b - a) - 256
                for o in range(0, b - a, 512):
                    w = min(o + 512, b - a) - o
                    last_bank = diag and (o + w == b - a)
                    nc.tensor.matmul(ps[:, o:o + w], kt_k,
                                     qt_bf[:, a + o:a + o + w],
                                     start=True, stop=not last_bank)
                if diag:
                    nc.tensor.matmul(ps[:, m0:m0 + 256], id_bf, mask_bf,
                                     start=False, stop=True)
                acc = small.tile([128, 1], f32, tag="acc")
                nc.scalar.activation(out=e_all[:, S + a:S + b],
                                     in_=ps[:, 0:b - a],
                                     func=AF.Exp, scale=0.125, accum_out=acc)
                accs.append(acc)

            # V projection for this tile (1-bank PSUM, consumed by vs mul)
            v_ps = vv.tile([128, 256], f32, tag="vv")
            nc.tensor.matmul(v_ps, xj_f[:, k * 128:(k + 1) * 128],
                             wv_p[:], start=True, stop=False)
            nc.tensor.matmul(v_ps, ones_s[:],
                             bv_p[:], start=False, stop=True)

            # read burst deferred from the previous strip boundary: PE works
            # on it while ACT chews this strip's exps
            for (i0, i1, kmin) in pending:
                if kmin == 0:
                    # split: k'>=1 now, k'=0 after vs[0] exists (below)
                    ps_last = read_pass(i0, i1, 0, klo=1)
                else:
                    drain_pass(read_pass(i0, i1, kmin), i0, i1)
            had_final = any(kmin == 0 for (_, _, kmin) in pending)
            pending = [p for p in _PASSES if p[2] == k]

            s_t = accs[0]
            for extra in accs[1:]:
                s2 = small.tile([128, 1], f32, tag="s")
                nc.vector.tensor_add(s2, s_t, extra)
                s_t = s2
            rs = small.tile([128, 1], f32, tag="rs")
            nc.vector.reciprocal(rs, s_t)
            nc.vector.tensor_scalar_mul(vs_bf[:, k, :], v_ps[:, 0:_V], rs)

        # tail: the final (0,256) pass — only k'=0 remains
        (i0, i1, _) = _PASSES[-1]
        ps_last = read_pass(i0, i1, 0, klo=1)
        nc.tensor.matmul(ps_last[0:_V, 0:256], vs_bf[:, 0, :],
                         e_all[:, _S(0) + i0:_S(0) + i0 + 256],
                         start=False, stop=True)
        drain_pass(ps_last, i0, i1)

    nc.compile()
    return nc


def _get_nc():
    if "nc" not in _cache:
        _cache["nc"] = _build_nc()
    return _cache["nc"]


def _masks(g):
    """Additive mask for the last 256 columns of every strip.

    Strip for local tile k covers i in [0, (2k+2)*128); its last 256
    columns are i = 2k*128 + u, u in [0, 256). Partition p holds global
    j = (2k+g)*128 + p, so live (i <= j) iff u <= 128*g + p.
    """
    m = np.zeros((128, 256), np.float32)
    p = np.arange(128)[:, None]
    u = np.arange(256)[None, :]
    m[:] = np.where(u <= 128 * g + p, 0.0, _NEG)
    return m


def kernel(**inputs):
    from concourse.bass_utils import run_bass_kernel_spmd

    x = np.ascontiguousarray(np.asarray(inputs["x"], dtype=np.float32))
    Wq = np.ascontiguousarray(np.asarray(inputs["Wq"], dtype=np.float32))
    Wk = np.ascontiguousarray(np.asarray(inputs["Wk"], dtype=np.float32))
    Wv = np.ascontiguousarray(np.asarray(inputs["Wv"], dtype=np.float32))
    bq = np.ascontiguousarray(
        np.asarray(inputs["bq"], dtype=np.float32).reshape(_K, 1))
    bk = np.ascontiguousarray(
        np.asarray(inputs["bk"], dtype=np.float32).reshape(_K, 1))
    bv = np.zeros((1, 256), np.float32)
    bv[0, :_V] = np.asarray(inputs["bv"], dtype=np.float32).ravel()
    Wv_p = np.zeros((_C, 256), np.float32)
    Wv_p[:, :_V] = Wv
    ones = np.ones((1, 128), np.float32)

    nc = _get_nc()
    in_maps = []
    for core in range(8):
        b, g = divmod(core, 2)
        # this core's j columns: global tiles {2k+g}, i.e. starts 256k+128g
        cols = ((np.arange(_JT) * 256 + 128 * g)[:, None]
                + np.arange(128)[None, :]).ravel()
        in_maps.append({
            "xb": np.ascontiguousarray(x[b]),
            "xj": np.ascontiguousarray(x[b][:, cols]),
            "wq": Wq, "wk": Wk, "wv": Wv_p, "ones": ones,
            "bq": bq, "bk": bk, "bv": bv,
            "mask": _masks(g),
        })

    trace = bool(_cache.get("trace"))
    res = run_bass_kernel_spmd(nc, in_maps, core_ids=list(range(8)),
                               trace=trace)
    _cache["last_result"] = res

    parts = [r["out"] for r in res.results]
    out = np.empty((_B, _C + _V, _T), np.float32)
    for b in range(_B):
        out[b, :_C] = x[b]
        out[b, _C:] = parts[2 * b] + parts[2 * b + 1]
    return out
